# revision 54
# baseline (speedup 1.0000x reference)
"""Dense 2-layer GAT on 8 Trainium2 NeuronCores (Bass/Tile, SPMD) — v2.

Problem: B=4 graphs, N=2048 nodes, F_in=128, H=8 heads, F_hid=64, C=32.
Sharding: 2 cores per graph, each core owns 1024 attention rows (r-shard)
for all heads in layer 1 and for layer 2; a [1024,33] AllGather of Wh2
crosses cores between the layers.

v2 changes vs v1 (all driven by the DVE bottleneck: stt ptr-scalar ops
run at 1 elem/cycle while tensor_scalar-with-AP-scalar keeps the 4x DVE
perf mode and tensor_tensor keeps 2x; cost-model sim: 428us -> 305us):
  - u-build split into ts-add(f1rep, f2_ap)@4x + tt-add(madj)@2x instead
    of one stt@1x.
  - leaky-relu split into ts-mult@4x + tt-max@2x on DVE, or fused with
    the f2-add on ACT via Prelu's per-partition bias operand ('A' sites
    in SCHED; balances DVE ~225us vs ACT ~215us busy).
  - setup matmuls in float32r (1 cycle/row vs 4 for fp32): PE busy
    154us -> 86us and a shorter critical path to the first site.
  - softmax epilogue batched per head-pair on one [128,2048] PSUM tile;
    reciprocal via DVE reciprocal_approx_fast (full-partition APs only —
    partition-offset slices of custom-DVE ops mis-address); split ELU.
  - AllGather payload trimmed 65->33 cols (ones rebuilt locally); L2
    u1 = g1+mask prebuilt during the collective; L2 all-D post-cc.
HW quirks found on this toolchain (device-verified):
  - Act.Lrelu IGNORES alpha (computes relu); Act.Prelu honors it.
  - Pool/GPSIMD rejects TensorTensor/TensorScalar[Ptr] at codegen
    (engine check); only memsets/collectives/DMA go there.
  - reciprocal_approx_fast corrupts on partition-offset input slices.
"""
import os
import numpy as np
import ml_dtypes

import concourse.bass as bass
import concourse.mybir as mybir
import concourse.tile as tile
from concourse.bass_utils import run_bass_kernel_spmd
from concourse.vector_clock import VectorClock, ScopedClock

F32 = mybir.dt.float32
F32R = mybir.dt.float32r
F16 = mybir.dt.float16
Alu = mybir.AluOpType
Act = mybir.ActivationFunctionType

B, N, F_IN, H, F_HID, C = 4, 2048, 128, 8, 64, 32
RSH = N // 2           # rows per core
NCB = N // 128         # 16 c-chunks
ALPHA = 0.2
OUT_SLOPE = 0.01
MASK = -150.0
GROUPS = [[0, 1], [2, 3], [4, 5], [6, 7]]

# Per-site path schedule: 72 sites (64 L1 head-pair x chunk + 8 L2
# chunk-pairs).  'D' = all-DVE (ts-add@4x + tt-add@2x + split lrelu),
# 'A' = ACT-fused (tt-add@2x + Lrelu-with-f2-bias + exp on ACT).
# Pool/GPSIMD cannot run TensorTensor/TensorScalar (walrus engine check),
# so only memsets and the collective go there.
_L1A = "DADADADADADADADA"   # 8 ACT-fused sites per 16
SCHED = os.environ.get("GAT_SCHED", _L1A * 4 + "DDDDDDDD")

# ---------------------------------------------------------------------------
# Patch: Tile's kernel-tail drain aggregates one wait per outstanding proc
# lane into a single Drain instruction; walrus codegen only supports one
# sync wait per instruction ("Too many sync wait commands").  Split into a
# chain of single-wait drains instead.
import concourse.tile as _tile_mod

_ORIG_DRAIN = _tile_mod.TileContext._drain_and_barrier


def _split_drain_and_barrier(self, tick_clock, wait_clock):
    vals = eval(repr(tick_clock.global_clock).split("VectorClock(", 1)[1].rstrip(")"))
    for i, v in enumerate(vals):
        if v <= 0:
            continue
        part = VectorClock()
        part.require_at_least(i, v)
        d = self.nc.sync.drain()
        wait_clock.add_sem_waits(d.ins, ScopedClock({None: part}))
    self.nc.sync.drain()
    self.nc.all_engine_barrier()
    popped = self.nc._tile_sem_poison_stack.pop()
    assert popped is self._sem_poison
    self.nc.clear_and_free_semaphores(list(self.sems.allocated().values()))
    self.nc.all_engine_barrier()


_tile_mod.TileContext._drain_and_barrier = _split_drain_and_barrier

def _legalize_multi_waits(nc):
    """Walrus codegen accepts at most one sync wait per instruction; hoist
    extra waits onto preceding same-engine sequencer NOPs."""
    Op = nc.isa.Opcode

    def mk_nop(engine):
        return nc.engines[engine]._isa(Op.NEURON_ISA_TPB_OPCODE_NOP, {})

    n_fix = 0
    for f in nc.m.functions:
        for bb in f.blocks:
            insts = list(bb.instructions)
            if not any(i.sync_info and i.sync_info.on_wait
                       and len(i.sync_info.on_wait) > 1 for i in insts):
                continue
            new = []
            for inst in insts:
                si = inst.sync_info
                if si and si.on_wait and len(si.on_wait) > 1:
                    waits = list(si.on_wait)
                    for w in waits[:-1]:
                        nop = mk_nop(inst.engine)
                        nop.sync_info = mybir.SyncInfo(on_wait=[w], on_update=[])
                        new.append(nop)
                        n_fix += 1
                    inst.sync_info = mybir.SyncInfo(
                        on_wait=[waits[-1]], on_update=list(si.on_update or []))
                new.append(inst)
            bb.instructions = new
    return n_fix
# ---------------------------------------------------------------------------


def build_nc(loops=1):
    """loops>1 repeats the whole kernel body inside the program — used by
    test.py to measure per-execution device time by (T(N)-T(1))/(N-1),
    which cancels the axon dispatch overhead exactly."""
    nc = bass.Bass(num_devices=8)

    xT_e = nc.dram_tensor("xT", [F_IN, N], F32R, kind="ExternalInput")
    xtr_e = nc.dram_tensor("xtr", [F_IN, RSH], F32R, kind="ExternalInput")
    madj_e = nc.dram_tensor("madj", [N, RSH], F16, kind="ExternalInput")
    wall_e = nc.dram_tensor("wall", [F_IN, H * F_HID], F32R, kind="ExternalInput")
    wa2_e = nc.dram_tensor("wa2", [F_IN, H], F32R, kind="ExternalInput")
    w1rep_e = nc.dram_tensor("w1rep", [F_IN, H * 128], F32R, kind="ExternalInput")
    woaug_e = nc.dram_tensor("woaug", [H * F_HID, 33], F16, kind="ExternalInput")
    wo1rep_e = nc.dram_tensor("wo1rep", [H * F_HID, 128], F16, kind="ExternalInput")
    out_e = nc.dram_tensor("out", [C, RSH], F32, kind="ExternalOutput")
    cc_in = nc.dram_tensor("cc_in", [RSH, 33], F16)
    cc_out = nc.dram_tensor("cc_out", [N, 33], F16)
    DBG = os.environ.get("GAT_DEBUG", "0") == "1"
    if DBG:
        dbg = {}
        for nm, shp, dt in [("dbg_hT0", [128, RSH], F16), ("dbg_hT3", [128, RSH], F16),
                            ("dbg_g1rep", [128, RSH], F16), ("dbg_wh2_0", [128, 64], F16),
                            ("dbg_g2sb", [128, NCB], F32), ("dbg_f1rep0", [128, RSH], F16),
                            ("dbg_f2sb", [128, NCB * H], F32), ("dbg_whaug0", [128, H * 128], F16),
                            ("dbg_rinv0", [128, RSH * 2], F32), ("dbg_hn0", [64, RSH * 2], F16)]:
            dbg[nm] = nc.dram_tensor(nm, shp, dt, kind="ExternalOutput")
        dbg["dbg_ph1_0"] = nc.dram_tensor("dbg_ph1_0", [128, RSH * 2], F32,
                                          kind="ExternalOutput")
        dbg["dbg_p00"] = nc.dram_tensor("dbg_p00", [128, RSH * 2], F16,
                                        kind="ExternalOutput")
        dbg["dbg_u00"] = nc.dram_tensor("dbg_u00", [128, RSH * 2], F16,
                                        kind="ExternalOutput")

    with tile.TileContext(nc) as tc:
        from contextlib import ExitStack
        for _loop_it in range(loops):
          with ExitStack() as ctx:
            res = ctx.enter_context(tc.tile_pool(name="res", bufs=1))
            work = ctx.enter_context(tc.tile_pool(name="work", bufs=5))
            ep = ctx.enter_context(tc.tile_pool(name="ep", bufs=1))
            setup_cm = tc.tile_pool(name="setup", bufs=1)
            setup = setup_cm.__enter__()

            # ---------------- input loads ----------------
            # setup-critical loads on the SP queue, halved so two HWDGE
            # queues transfer in parallel; bulk (madj) and late-phase
            # weights go via the ACT queue to keep SP issue short.
            # float32r: tf32-class matmul mode, 1 cycle/row (vs 4 for fp32)
            xT = setup.tile([F_IN, N], F32R, tag="xT")
            nc.sync.dma_start(out=xT[:, 0:N // 2], in_=xT_e[:, 0:N // 2])
            nc.sync.dma_start(out=xT[:, N // 2:], in_=xT_e[:, N // 2:])
            wall = setup.tile([F_IN, H * F_HID], F32R, tag="wall")
            nc.sync.dma_start(out=wall, in_=wall_e[:, :])
            w1rep = setup.tile([F_IN, H * 128], F32R, tag="w1rep")
            nc.sync.dma_start(out=w1rep[:, 0:512], in_=w1rep_e[:, 0:512])
            nc.sync.dma_start(out=w1rep[:, 512:], in_=w1rep_e[:, 512:])
            xtr = setup.tile([F_IN, RSH], F32R, tag="xtr")
            nc.sync.dma_start(out=xtr, in_=xtr_e[:, :])
            wa2 = setup.tile([F_IN, H], F32R, tag="wa2")
            nc.sync.dma_start(out=wa2, in_=wa2_e[:, :])
            madj = []
            for cb in range(NCB):
                t = res.tile([128, RSH], F16, tag=f"madj{cb}", name=f"madj{cb}")
                nc.sync.dma_start(out=t, in_=madj_e[cb * 128:(cb + 1) * 128, :])
                madj.append(t)
            # needed only from the exchange phase on
            woaug = [res.tile([128, 33], F16, tag=f"woaug{k}", name=f"woaug{k}") for k in range(4)]
            wo1rep = [res.tile([128, 128], F16, tag=f"wo1rep{k}", name=f"wo1rep{k}") for k in range(4)]
            for k in range(4):
                nc.sync.dma_start(out=woaug[k], in_=woaug_e[k * 128:(k + 1) * 128, :])
                nc.sync.dma_start(out=wo1rep[k], in_=wo1rep_e[k * 128:(k + 1) * 128, :])

            whaug = [res.tile([128, H * 128], F16, tag=f"whaug{cb}", name=f"whaug{cb}") for cb in range(NCB)]
            f1rep = [res.tile([128, RSH], F16, tag=f"f1rep{h}", name=f"f1rep{h}") for h in range(H)]
            f2sb = res.tile([128, NCB * H], F32, tag="f2sb")
            hT = [res.tile([128, RSH], F16, tag=f"hT{k}", name=f"hT{k}") for k in range(4)]

            with tc.tile_pool(name="ps_set", bufs=2, space="PSUM") as ps_set:
                # f1 (head pair 0 first): site (0,*) u-builds gate on f1rep[0:2]
                # and f2sb, so emit those before the bulk Wh chunks.
                for h in range(2):
                    pf1 = ps_set.tile([128, RSH], F32, tag="set_f1")
                    for j in range(2):
                        nc.tensor.matmul(pf1[:, j * 512:(j + 1) * 512],
                                         lhsT=w1rep[:, h * 128:(h + 1) * 128],
                                         rhs=xtr[:, j * 512:(j + 1) * 512],
                                         start=True, stop=True)
                    nc.scalar.activation(out=f1rep[h], in_=pf1, func=Act.Copy)
                # Wh per c-chunk: [128, 512] = all heads side by side
                for cb in range(NCB):
                    pwh = ps_set.tile([128, H * F_HID], F32, tag="set_a")
                    nc.tensor.matmul(pwh, lhsT=xT[:, cb * 128:(cb + 1) * 128],
                                     rhs=wall, start=True, stop=True)
                    # strided copy into whaug (64 Wh cols of each 128-col head block)
                    wh_v = whaug[cb].rearrange("p (hh q) -> p hh q", q=128)
                    dst = wh_v[:, :, 0:F_HID]
                    src = pwh.rearrange("p (hh o) -> p hh o", o=F_HID)
                    # ACT takes all PSUM->SBUF copies: DVE is the scarcer
                    # engine (236us vs 211us busy in sim)
                    nc.scalar.activation(out=dst, in_=src, func=Act.Copy)
                    nc.gpsimd.memset(wh_v[:, :, F_HID:128], 1.0)

                    # f2 for this chunk: [128, H]
                    pf2 = ps_set.tile([128, H], F32, tag="set_b")
                    nc.tensor.matmul(pf2, lhsT=xT[:, cb * 128:(cb + 1) * 128],
                                     rhs=wa2, start=True, stop=True)
                    nc.vector.tensor_copy(out=f2sb[:, cb * H:(cb + 1) * H], in_=pf2)

                # remaining heads' f1
                for h in range(2, H):
                    pf1 = ps_set.tile([128, RSH], F32, tag="set_f1")
                    for j in range(2):
                        nc.tensor.matmul(pf1[:, j * 512:(j + 1) * 512],
                                         lhsT=w1rep[:, h * 128:(h + 1) * 128],
                                         rhs=xtr[:, j * 512:(j + 1) * 512],
                                         start=True, stop=True)
                    nc.scalar.activation(out=f1rep[h], in_=pf1, func=Act.Copy)
            setup_cm.__exit__(None, None, None)

            def build_site(site, u, scal_aps, mask_t, heads_in):
                """u[:, i*RSH:(i+1)*RSH] = exp(lrelu(in_i + scal_i + mask)).
                Writes p (exp output) into a fresh work tile; returns it.
                heads_in: two [128, RSH] f16 tiles (f1rep[h] or g1rep).
                scal_aps: two [128,1] scalar APs added per partition."""
                path = SCHED[site % len(SCHED)]
                if path == "A":
                    for i in range(2):
                        sl = u[:, i * RSH:(i + 1) * RSH]
                        nc.vector.tensor_tensor(out=sl, in0=heads_in[i], in1=mask_t,
                                                op=Alu.add)
                        nc.scalar.activation(out=sl, in_=sl, func=Act.Prelu,
                                             bias=scal_aps[i], alpha=ALPHA)
                else:
                    for i in range(2):
                        sl = u[:, i * RSH:(i + 1) * RSH]
                        nc.vector.tensor_scalar(out=sl, in0=heads_in[i],
                                                scalar1=scal_aps[i], scalar2=None,
                                                op0=Alu.add)
                        nc.vector.tensor_tensor(out=sl, in0=sl, in1=mask_t,
                                                op=Alu.add)
                    p = work.tile([128, RSH * 2], F16, tag="p")
                    nc.vector.tensor_scalar(out=p, in0=u, scalar1=ALPHA,
                                            scalar2=None, op0=Alu.mult)
                    nc.vector.tensor_tensor(out=u, in0=u, in1=p, op=Alu.max)
                    nc.scalar.activation(out=p, in_=u, func=Act.Exp)
                    return p
                p = work.tile([128, RSH * 2], F16, tag="p")
                nc.scalar.activation(out=p, in_=u, func=Act.Exp)
                return p

            with tc.tile_pool(name="ps_main", bufs=1, space="PSUM") as ps_main:
                # PSUM is 8 banks = 16KB/partition; a [128,2048] f32 pair tile
                # is 4 banks.  Two alternating 1-buf tags give double-buffering
                # within budget, and the later phases reuse the same rings.
                # ---------------- layer 1 ----------------
                for hp in range(H // 2):
                    ha, hb = 2 * hp, 2 * hp + 1
                    ph1 = ps_main.tile([128, RSH * 2], F32,
                                       tag="pa" if hp % 2 == 0 else "pb",
                                       name=f"ph1_{hp}")
                    for cb in range(NCB):
                        u = work.tile([128, RSH * 2], F16, tag="u")
                        p = build_site(hp * NCB + cb, u,
                                       [f2sb[:, cb * H + ha:cb * H + ha + 1],
                                        f2sb[:, cb * H + hb:cb * H + hb + 1]],
                                       madj[cb], [f1rep[ha], f1rep[hb]])
                        if DBG and hp == 0 and cb == 0:
                            nc.sync.dma_start(out=dbg["dbg_p00"][:, :], in_=p)
                            nc.sync.dma_start(out=dbg["dbg_u00"][:, :], in_=u)
                        for i in range(2):
                            h = ha + i
                            for j in range(2):
                                nc.tensor.matmul(
                                    ph1[:, i * RSH + j * 512:i * RSH + (j + 1) * 512],
                                    lhsT=whaug[cb][:, h * 128:(h + 1) * 128],
                                    rhs=p[:, i * RSH + j * 512:i * RSH + (j + 1) * 512],
                                    start=(cb == 0), stop=(cb == NCB - 1))
                    # epilogue, pair-batched on [64, 2048]:
                    # rows 64:128 of each head block hold the replicated row-sum.
                    # recip/shift/hn pipelined in 1024-col quarters to shorten
                    # the serial chain after the last matmul.
                    if DBG and hp == 0:
                        pcp = ep.tile([128, RSH * 2], F32, tag="rinv")
                        nc.vector.tensor_copy(out=pcp, in_=ph1)
                        nc.sync.dma_start(out=dbg["dbg_ph1_0"][:, :], in_=pcp)
                    rinv = ep.tile([128, RSH * 2], F32, tag="rinv")
                    hn = ep.tile([64, RSH * 2], F16, tag="hn")
                    for q4 in range(2):
                        qs = slice(q4 * RSH, (q4 + 1) * RSH)
                        # full-partition AP: reciprocal_approx_fast mis-addresses
                        # partition-offset slices (rows 0:64 are don't-care)
                        nc.vector.reciprocal_approx_fast(out=rinv[:, qs],
                                                         in_=ph1[:, qs])
                        nc.sync.dma_start(out=rinv[0:64, qs], in_=rinv[64:128, qs])
                        nc.vector.tensor_tensor(out=hn[:, qs], in0=ph1[0:64, qs],
                                                in1=rinv[0:64, qs], op=Alu.mult)
                    # ELU: q = exp(min(hn,0)); h' = max(q-1, hn)
                    q = ep.tile([64, RSH * 2], F16, tag="q")
                    nc.vector.tensor_scalar(out=q, in0=hn, scalar1=0.0,
                                            scalar2=None, op0=Alu.min)
                    nc.scalar.activation(out=q, in_=q, func=Act.Exp)
                    nc.vector.tensor_scalar(out=q, in0=q, scalar1=-1.0,
                                            scalar2=None, op0=Alu.add)
                    nc.vector.tensor_tensor(out=hT[hp][0:64, :], in0=q[:, 0:RSH],
                                            in1=hn[:, 0:RSH], op=Alu.max)
                    tmp = ep.tile([64, RSH], F16, tag="hodd")
                    nc.vector.tensor_tensor(out=tmp, in0=q[:, RSH:],
                                            in1=hn[:, RSH:], op=Alu.max)
                    nc.sync.dma_start(out=hT[hp][64:128, :], in_=tmp)
                    if DBG and hp == 0:
                        nc.sync.dma_start(out=dbg["dbg_rinv0"][:, :], in_=rinv)
                        nc.sync.dma_start(out=dbg["dbg_hn0"][:, :], in_=hn)

                # ---------------- Wh2 + exchange ----------------
                ccsb = res.tile([128, 8 * 33], F16, tag="ccsb")
                for half in range(2):
                    pw2 = ps_main.tile([128, 4 * 33], F32,
                                       tag="pa" if half == 0 else "pb")
                    for nbq in range(4):
                        nb = half * 4 + nbq
                        for k in range(4):
                            nc.tensor.matmul(pw2[:, nbq * 33:(nbq + 1) * 33],
                                             lhsT=hT[k][:, nb * 128:(nb + 1) * 128],
                                             rhs=woaug[k], start=(k == 0), stop=(k == 3))
                    nc.vector.tensor_copy(
                        out=ccsb[:, half * 132:(half + 1) * 132], in_=pw2)
                nc.sync.dma_start(
                    out=cc_in[:, :].rearrange("(nb p) j -> p nb j", p=128),
                    in_=ccsb.rearrange("p (nb j) -> p nb j", j=33))
                nc.gpsimd.collective_compute(
                    "AllGather", Alu.bypass, replica_groups=GROUPS,
                    ins=[cc_in[:, :]], outs=[cc_out[:, :]])

                # g1 replicated (no collective dependency): [128, 1024]
                pg1 = ps_main.tile([128, RSH], F32, tag="pa")
                for j in range(2):
                    for k in range(4):
                        nc.tensor.matmul(pg1[:, j * 512:(j + 1) * 512],
                                         lhsT=wo1rep[k],
                                         rhs=hT[k][:, j * 512:(j + 1) * 512],
                                         start=(k == 0), stop=(k == 3))
                g1rep = res.tile([128, RSH], F16, tag="g1rep")
                nc.vector.tensor_copy(out=g1rep, in_=pg1)
                # prebuild L2 u1 = g1 + mask for the first 6 sites while the
                # AllGather is in flight (no cc dependency -> DVE stays busy);
                # the last 2 build inline post-cc, freeing 8KB/partition of
                # SBUF for a deeper work-tile ring.
                N_PRE = 6
                l2u = []
                for cbp in range(N_PRE):
                    u2 = work.tile([128, RSH * 2], F16, tag="l2u", bufs=N_PRE,
                                   name=f"l2u{cbp}")
                    for i, cc in enumerate((2 * cbp, 2 * cbp + 1)):
                        nc.vector.tensor_tensor(out=u2[:, i * RSH:(i + 1) * RSH],
                                                in0=g1rep, in1=madj[cc], op=Alu.add)
                    l2u.append(u2)
                # wh2[cb]: [Wh2(32) | ones(32)]; g2 lands in one [128,16] tile
                wh2 = [res.tile([128, 64], F16, tag=f"wh2_{cb}", name=f"wh2_{cb}") for cb in range(NCB)]
                cc_out_r = cc_out[:, :].rearrange("(cb p) j -> p cb j", p=128)
                g2f16 = res.tile([128, NCB], F16, tag="g2f16")
                nc.sync.dma_start(out=g2f16, in_=cc_out_r[:, :, 32])
                g2sb = res.tile([128, NCB], F32, tag="g2sb")
                nc.vector.tensor_copy(out=g2sb, in_=g2f16)
                for cb in range(NCB):
                    nc.sync.dma_start(out=wh2[cb][:, 0:32], in_=cc_out_r[:, cb, 0:32])
                    nc.gpsimd.memset(wh2[cb][:, 32:64], 1.0)

                if DBG:
                    nc.sync.dma_start(out=dbg["dbg_hT0"][:, :], in_=hT[0])
                    nc.sync.dma_start(out=dbg["dbg_hT3"][:, :], in_=hT[3])
                    nc.sync.dma_start(out=dbg["dbg_g1rep"][:, :], in_=g1rep)
                    nc.sync.dma_start(out=dbg["dbg_wh2_0"][:, :], in_=wh2[0])
                    nc.sync.dma_start(out=dbg["dbg_g2sb"][:, :], in_=g2sb)
                    nc.sync.dma_start(out=dbg["dbg_f1rep0"][:, :], in_=f1rep[0])
                    nc.sync.dma_start(out=dbg["dbg_f2sb"][:, :], in_=f2sb)
                    nc.sync.dma_start(out=dbg["dbg_whaug0"][:, :], in_=whaug[0])

                # ---------------- layer 2 ----------------
                po = ps_main.tile([128, RSH], F32, tag="pb")
                for cbp in range(NCB // 2):
                    ca, cb2 = 2 * cbp, 2 * cbp + 1
                    if cbp < N_PRE:
                        u2 = l2u[cbp]
                    else:
                        u2 = work.tile([128, RSH * 2], F16, tag="u")
                        for i, cc in enumerate((ca, cb2)):
                            nc.vector.tensor_tensor(
                                out=u2[:, i * RSH:(i + 1) * RSH],
                                in0=g1rep, in1=madj[cc], op=Alu.add)
                    path = SCHED[(64 + cbp) % len(SCHED)]
                    if path == "A":
                        for i, cc in enumerate((ca, cb2)):
                            sl = u2[:, i * RSH:(i + 1) * RSH]
                            nc.scalar.activation(out=sl, in_=sl, func=Act.Prelu,
                                                 bias=g2sb[:, cc:cc + 1], alpha=ALPHA)
                        p2 = work.tile([128, RSH * 2], F16, tag="p")
                        nc.scalar.activation(out=p2, in_=u2, func=Act.Exp)
                    else:
                        for i, cc in enumerate((ca, cb2)):
                            sl = u2[:, i * RSH:(i + 1) * RSH]
                            nc.vector.tensor_scalar(out=sl, in0=sl,
                                                    scalar1=g2sb[:, cc:cc + 1],
                                                    scalar2=None, op0=Alu.add)
                        p2 = work.tile([128, RSH * 2], F16, tag="p")
                        nc.vector.tensor_scalar(out=p2, in0=u2, scalar1=ALPHA,
                                                scalar2=None, op0=Alu.mult)
                        nc.vector.tensor_tensor(out=u2, in0=u2, in1=p2, op=Alu.max)
                        nc.scalar.activation(out=p2, in_=u2, func=Act.Exp)
                    for i, cc in enumerate((ca, cb2)):
                        for j in range(2):
                            nc.tensor.matmul(
                                po[0:64, j * 512:(j + 1) * 512],
                                lhsT=wh2[cc],
                                rhs=p2[:, i * RSH + j * 512:i * RSH + (j + 1) * 512],
                                start=(cc == 0), stop=(cc == NCB - 1))
                # L2 epilogue: rows 32:64 hold replicated sums
                rinv2 = ep.tile([64, RSH], F32, tag="rinv2")
                ov = ep.tile([32, RSH], F32, tag="ov")
                osb = ep.tile([32, RSH], F32, tag="osb")
                nc.vector.reciprocal_approx_fast(out=rinv2, in_=po[0:64, :])
                nc.sync.dma_start(out=rinv2[0:32, :], in_=rinv2[32:64, :])
                nc.vector.tensor_tensor(out=ov, in0=po[0:32, :],
                                        in1=rinv2[0:32, :], op=Alu.mult)
                nc.vector.tensor_scalar(out=osb, in0=ov, scalar1=OUT_SLOPE,
                                        scalar2=None, op0=Alu.mult)
                nc.vector.tensor_tensor(out=osb, in0=osb, in1=ov, op=Alu.max)
                nc.sync.dma_start(out=out_e[:, :], in_=osb)
    from concourse.library_overlay import lower_extended_insts
    lower_extended_insts(nc)
    _legalize_multi_waits(nc)
    return nc


_NC = None


def _host_prep(x, adj, W, a1, a2, Wout, ao1, ao2):
    x = np.asarray(x, dtype=np.float32)
    adj = np.asarray(adj, dtype=np.float32)
    W = np.asarray(W, dtype=np.float32)
    a1 = np.asarray(a1, dtype=np.float32)
    a2 = np.asarray(a2, dtype=np.float32)
    Wout = np.asarray(Wout, dtype=np.float32)
    ao1 = np.asarray(ao1, dtype=np.float32)
    ao2 = np.asarray(ao2, dtype=np.float32)

    xT = np.ascontiguousarray(x.transpose(0, 2, 1))                # [B,128,N]
    madj = ((adj.transpose(0, 2, 1) - 1.0) * (-MASK)).astype(np.float16)  # 0 / -150, [B,N,N] as (c,r)
    wall = np.ascontiguousarray(W.transpose(1, 0, 2).reshape(F_IN, H * F_HID))
    wa1 = np.einsum('hfo,ho->fh', W, a1)                           # [128,H]
    wa2 = np.ascontiguousarray(np.einsum('hfo,ho->fh', W, a2))     # [128,H]
    w1rep = np.repeat(wa1, 128, axis=1)                            # [128,H*128]
    wo1 = Wout @ ao1                                               # [512]
    wo2 = Wout @ ao2
    woaug = np.zeros((H * F_HID, 33), np.float16)
    woaug[:, :C] = Wout.astype(np.float16)
    woaug[:, 32] = wo2.astype(np.float16)
    wo1rep = np.repeat(wo1[:, None], 128, axis=1).astype(np.float16)

    in_maps = []
    for c in range(8):
        b, s = c // 2, c % 2
        sl = slice(s * RSH, (s + 1) * RSH)
        in_maps.append({
            "xT": np.ascontiguousarray(xT[b]),
            "xtr": np.ascontiguousarray(xT[b][:, sl]),
            "madj": np.ascontiguousarray(madj[b][:, sl]),
            "wall": wall,
            "wa2": wa2,
            "w1rep": np.ascontiguousarray(w1rep),
            "woaug": woaug,
            "wo1rep": wo1rep,
        })
    return in_maps


def run(x, adj, W, a1, a2, Wout, ao1, ao2, trace=False, **trace_kw):
    global _NC
    if _NC is None:
        _NC = build_nc()
    in_maps = _host_prep(x, adj, W, a1, a2, Wout, ao1, ao2)
    r = run_bass_kernel_spmd(_NC, in_maps, list(range(8)), trace=trace, **trace_kw)
    out = np.empty((B, N, C), np.float32)
    for c in range(8):
        b, s = c // 2, c % 2
        out[b, s * RSH:(s + 1) * RSH, :] = r.results[c]["out"].T
    return out, r


def kernel(x, adj, W, a1, a2, Wout, ao1, ao2, batch_size=None):
    out, _ = run(x, adj, W, a1, a2, Wout, ao1, ao2)
    return out


# revision 55
# speedup vs baseline: 1.5715x; 1.5715x over previous
"""Dense 2-layer GAT on 8 Trainium2 NeuronCores (Bass/Tile, SPMD) — v2.

Problem: B=4 graphs, N=2048 nodes, F_in=128, H=8 heads, F_hid=64, C=32.
Sharding: 2 cores per graph, each core owns 1024 attention rows (r-shard)
for all heads in layer 1 and for layer 2; a [1024,33] AllGather of Wh2
crosses cores between the layers.

v2 changes vs v1 (all driven by the DVE bottleneck: stt ptr-scalar ops
run at 1 elem/cycle while tensor_scalar-with-AP-scalar keeps the 4x DVE
perf mode and tensor_tensor keeps 2x; cost-model sim: 428us -> 305us):
  - u-build split into ts-add(f1rep, f2_ap)@4x + tt-add(madj)@2x instead
    of one stt@1x.
  - leaky-relu split into ts-mult@4x + tt-max@2x on DVE, or fused with
    the f2-add on ACT via Prelu's per-partition bias operand ('A' sites
    in SCHED; balances DVE ~225us vs ACT ~215us busy).
  - setup matmuls in float32r (1 cycle/row vs 4 for fp32): PE busy
    154us -> 86us and a shorter critical path to the first site.
  - softmax epilogue batched per head-pair on one [128,2048] PSUM tile;
    reciprocal via DVE reciprocal_approx_fast (full-partition APs only —
    partition-offset slices of custom-DVE ops mis-address); split ELU.
  - AllGather payload trimmed 65->33 cols (ones rebuilt locally); L2
    u1 = g1+mask prebuilt during the collective; L2 all-D post-cc.
HW quirks found on this toolchain (device-verified):
  - Act.Lrelu IGNORES alpha (computes relu); Act.Prelu honors it.
  - Pool/GPSIMD rejects TensorTensor/TensorScalar[Ptr] at codegen
    (engine check); only memsets/collectives/DMA go there.
  - reciprocal_approx_fast corrupts on partition-offset input slices.
"""
import os
import numpy as np
import ml_dtypes

import concourse.bass as bass
import concourse.mybir as mybir
import concourse.tile as tile
from concourse.bass_utils import run_bass_kernel_spmd
from concourse.vector_clock import VectorClock, ScopedClock

F32 = mybir.dt.float32
F32R = mybir.dt.float32r
F16 = mybir.dt.float16
Alu = mybir.AluOpType
Act = mybir.ActivationFunctionType

B, N, F_IN, H, F_HID, C = 4, 2048, 128, 8, 64, 32
RSH = N // 2           # rows per core
NCB = N // 128         # 16 c-chunks
ALPHA = 0.2
OUT_SLOPE = 0.01
MASK = -150.0
GROUPS = [[0, 1], [2, 3], [4, 5], [6, 7]]

# Per-site path schedule: 72 sites (64 L1 head-pair x chunk + 8 L2
# chunk-pairs).  'D' = all-DVE (ts-add@4x + tt-add@2x + split lrelu),
# 'A' = ACT-fused (tt-add@2x + Lrelu-with-f2-bias + exp on ACT).
# Pool/GPSIMD cannot run TensorTensor/TensorScalar (walrus engine check),
# so only memsets and the collective go there.
_L1A = "DADADADADADADADA"   # 8 ACT-fused sites per 16
SCHED = os.environ.get("GAT_SCHED", _L1A * 4 + "DDDADDDA")

# ---------------------------------------------------------------------------
# Patch: Tile's kernel-tail drain aggregates one wait per outstanding proc
# lane into a single Drain instruction; walrus codegen only supports one
# sync wait per instruction ("Too many sync wait commands").  Split into a
# chain of single-wait drains instead.
import concourse.tile as _tile_mod

_ORIG_DRAIN = _tile_mod.TileContext._drain_and_barrier


def _split_drain_and_barrier(self, tick_clock, wait_clock):
    vals = eval(repr(tick_clock.global_clock).split("VectorClock(", 1)[1].rstrip(")"))
    for i, v in enumerate(vals):
        if v <= 0:
            continue
        part = VectorClock()
        part.require_at_least(i, v)
        d = self.nc.sync.drain()
        wait_clock.add_sem_waits(d.ins, ScopedClock({None: part}))
    self.nc.sync.drain()
    self.nc.all_engine_barrier()
    popped = self.nc._tile_sem_poison_stack.pop()
    assert popped is self._sem_poison
    self.nc.clear_and_free_semaphores(list(self.sems.allocated().values()))
    self.nc.all_engine_barrier()


_tile_mod.TileContext._drain_and_barrier = _split_drain_and_barrier

def _legalize_multi_waits(nc):
    """Walrus codegen accepts at most one sync wait per instruction; hoist
    extra waits onto preceding same-engine sequencer NOPs."""
    Op = nc.isa.Opcode

    def mk_nop(engine):
        return nc.engines[engine]._isa(Op.NEURON_ISA_TPB_OPCODE_NOP, {})

    n_fix = 0
    for f in nc.m.functions:
        for bb in f.blocks:
            insts = list(bb.instructions)
            if not any(i.sync_info and i.sync_info.on_wait
                       and len(i.sync_info.on_wait) > 1 for i in insts):
                continue
            new = []
            for inst in insts:
                si = inst.sync_info
                if si and si.on_wait and len(si.on_wait) > 1:
                    waits = list(si.on_wait)
                    for w in waits[:-1]:
                        nop = mk_nop(inst.engine)
                        nop.sync_info = mybir.SyncInfo(on_wait=[w], on_update=[])
                        new.append(nop)
                        n_fix += 1
                    inst.sync_info = mybir.SyncInfo(
                        on_wait=[waits[-1]], on_update=list(si.on_update or []))
                new.append(inst)
            bb.instructions = new
    return n_fix
# ---------------------------------------------------------------------------


def build_nc(loops=1):
    """loops>1 repeats the whole kernel body inside the program — used by
    test.py to measure per-execution device time by (T(N)-T(1))/(N-1),
    which cancels the axon dispatch overhead exactly."""
    nc = bass.Bass(num_devices=8)

    xT_e = nc.dram_tensor("xT", [F_IN, N], F32R, kind="ExternalInput")
    xtr_e = nc.dram_tensor("xtr", [F_IN, RSH], F32R, kind="ExternalInput")
    madj_e = nc.dram_tensor("madj", [N, RSH], F16, kind="ExternalInput")
    wall_e = nc.dram_tensor("wall", [F_IN, H * F_HID], F32R, kind="ExternalInput")
    wa2_e = nc.dram_tensor("wa2", [F_IN, H], F32R, kind="ExternalInput")
    w1rep_e = nc.dram_tensor("w1rep", [F_IN, H * 128], F32R, kind="ExternalInput")
    woaug_e = nc.dram_tensor("woaug", [H * F_HID, 33], F16, kind="ExternalInput")
    wo1rep_e = nc.dram_tensor("wo1rep", [H * F_HID, 128], F16, kind="ExternalInput")
    out_e = nc.dram_tensor("out", [C, RSH], F32, kind="ExternalOutput")
    cc_in = nc.dram_tensor("cc_in", [RSH, 33], F16)
    cc_out = nc.dram_tensor("cc_out", [N, 33], F16)
    DBG = os.environ.get("GAT_DEBUG", "0") == "1"
    if DBG:
        dbg = {}
        for nm, shp, dt in [("dbg_hT0", [128, RSH], F16), ("dbg_hT3", [128, RSH], F16),
                            ("dbg_g1rep", [128, RSH], F16), ("dbg_wh2_0", [128, 64], F16),
                            ("dbg_g2sb", [128, NCB], F32), ("dbg_f1rep0", [128, RSH], F16),
                            ("dbg_f2sb", [128, NCB * H], F32), ("dbg_whaug0", [128, H * 128], F16),
                            ("dbg_rinv0", [128, RSH * 2], F32), ("dbg_hn0", [64, RSH * 2], F16)]:
            dbg[nm] = nc.dram_tensor(nm, shp, dt, kind="ExternalOutput")
        dbg["dbg_ph1_0"] = nc.dram_tensor("dbg_ph1_0", [128, RSH * 2], F32,
                                          kind="ExternalOutput")
        dbg["dbg_p00"] = nc.dram_tensor("dbg_p00", [128, RSH * 2], F16,
                                        kind="ExternalOutput")
        dbg["dbg_u00"] = nc.dram_tensor("dbg_u00", [128, RSH * 2], F16,
                                        kind="ExternalOutput")

    with tile.TileContext(nc) as tc:
        from contextlib import ExitStack
        for _loop_it in range(loops):
          with ExitStack() as ctx:
            res = ctx.enter_context(tc.tile_pool(name="res", bufs=1))
            work = ctx.enter_context(tc.tile_pool(name="work", bufs=5))
            ep = ctx.enter_context(tc.tile_pool(name="ep", bufs=1))
            setup_cm = tc.tile_pool(name="setup", bufs=1)
            setup = setup_cm.__enter__()

            # ---------------- input loads ----------------
            # setup-critical loads on the SP queue, halved so two HWDGE
            # queues transfer in parallel; bulk (madj) and late-phase
            # weights go via the ACT queue to keep SP issue short.
            # float32r: tf32-class matmul mode, 1 cycle/row (vs 4 for fp32)
            xT = setup.tile([F_IN, N], F32R, tag="xT")
            nc.sync.dma_start(out=xT[:, 0:N // 2], in_=xT_e[:, 0:N // 2])
            nc.sync.dma_start(out=xT[:, N // 2:], in_=xT_e[:, N // 2:])
            wall = setup.tile([F_IN, H * F_HID], F32R, tag="wall")
            nc.sync.dma_start(out=wall, in_=wall_e[:, :])
            w1rep = setup.tile([F_IN, H * 128], F32R, tag="w1rep")
            nc.sync.dma_start(out=w1rep[:, 0:512], in_=w1rep_e[:, 0:512])
            nc.sync.dma_start(out=w1rep[:, 512:], in_=w1rep_e[:, 512:])
            xtr = setup.tile([F_IN, RSH], F32R, tag="xtr")
            nc.sync.dma_start(out=xtr, in_=xtr_e[:, :])
            wa2 = setup.tile([F_IN, H], F32R, tag="wa2")
            nc.sync.dma_start(out=wa2, in_=wa2_e[:, :])
            madj = []
            for cb in range(NCB):
                t = res.tile([128, RSH], F16, tag=f"madj{cb}", name=f"madj{cb}")
                nc.sync.dma_start(out=t, in_=madj_e[cb * 128:(cb + 1) * 128, :])
                madj.append(t)
            # needed only from the exchange phase on
            woaug = [res.tile([128, 33], F16, tag=f"woaug{k}", name=f"woaug{k}") for k in range(4)]
            wo1rep = [res.tile([128, 128], F16, tag=f"wo1rep{k}", name=f"wo1rep{k}") for k in range(4)]
            for k in range(4):
                nc.sync.dma_start(out=woaug[k], in_=woaug_e[k * 128:(k + 1) * 128, :])
                nc.sync.dma_start(out=wo1rep[k], in_=wo1rep_e[k * 128:(k + 1) * 128, :])

            whaug = [res.tile([128, H * 128], F16, tag=f"whaug{cb}", name=f"whaug{cb}") for cb in range(NCB)]
            f1rep = [res.tile([128, RSH], F16, tag=f"f1rep{h}", name=f"f1rep{h}") for h in range(H)]
            f2sb = res.tile([128, NCB * H], F32, tag="f2sb")
            hT = [res.tile([128, RSH], F16, tag=f"hT{k}", name=f"hT{k}") for k in range(4)]

            with tc.tile_pool(name="ps_set", bufs=2, space="PSUM") as ps_set:
                # f1 (head pair 0 first): site (0,*) u-builds gate on f1rep[0:2]
                # and f2sb, so emit those before the bulk Wh chunks.
                for h in range(2):
                    pf1 = ps_set.tile([128, RSH], F32, tag="set_f1")
                    for j in range(2):
                        nc.tensor.matmul(pf1[:, j * 512:(j + 1) * 512],
                                         lhsT=w1rep[:, h * 128:(h + 1) * 128],
                                         rhs=xtr[:, j * 512:(j + 1) * 512],
                                         start=True, stop=True)
                    nc.scalar.activation(out=f1rep[h], in_=pf1, func=Act.Copy)
                # Wh per c-chunk: [128, 512] = all heads side by side
                for cb in range(NCB):
                    pwh = ps_set.tile([128, H * F_HID], F32, tag="set_a")
                    nc.tensor.matmul(pwh, lhsT=xT[:, cb * 128:(cb + 1) * 128],
                                     rhs=wall, start=True, stop=True)
                    # strided copy into whaug (64 Wh cols of each 128-col head block)
                    wh_v = whaug[cb].rearrange("p (hh q) -> p hh q", q=128)
                    dst = wh_v[:, :, 0:F_HID]
                    src = pwh.rearrange("p (hh o) -> p hh o", o=F_HID)
                    # ACT takes all PSUM->SBUF copies: DVE is the scarcer
                    # engine (236us vs 211us busy in sim)
                    nc.scalar.activation(out=dst, in_=src, func=Act.Copy)
                    nc.gpsimd.memset(wh_v[:, :, F_HID:128], 1.0)

                    # f2 for this chunk: [128, H]
                    pf2 = ps_set.tile([128, H], F32, tag="set_b")
                    nc.tensor.matmul(pf2, lhsT=xT[:, cb * 128:(cb + 1) * 128],
                                     rhs=wa2, start=True, stop=True)
                    nc.vector.tensor_copy(out=f2sb[:, cb * H:(cb + 1) * H], in_=pf2)

                # remaining heads' f1
                for h in range(2, H):
                    pf1 = ps_set.tile([128, RSH], F32, tag="set_f1")
                    for j in range(2):
                        nc.tensor.matmul(pf1[:, j * 512:(j + 1) * 512],
                                         lhsT=w1rep[:, h * 128:(h + 1) * 128],
                                         rhs=xtr[:, j * 512:(j + 1) * 512],
                                         start=True, stop=True)
                    nc.scalar.activation(out=f1rep[h], in_=pf1, func=Act.Copy)
            setup_cm.__exit__(None, None, None)

            def build_site(site, u, scal_aps, mask_t, heads_in):
                """u[:, i*RSH:(i+1)*RSH] = exp(lrelu(in_i + scal_i + mask)).
                Writes p (exp output) into a fresh work tile; returns it.
                heads_in: two [128, RSH] f16 tiles (f1rep[h] or g1rep).
                scal_aps: two [128,1] scalar APs added per partition."""
                path = SCHED[site % len(SCHED)]
                if path == "A":
                    for i in range(2):
                        sl = u[:, i * RSH:(i + 1) * RSH]
                        nc.vector.tensor_tensor(out=sl, in0=heads_in[i], in1=mask_t,
                                                op=Alu.add)
                        nc.scalar.activation(out=sl, in_=sl, func=Act.Prelu,
                                             bias=scal_aps[i], alpha=ALPHA)
                else:
                    for i in range(2):
                        sl = u[:, i * RSH:(i + 1) * RSH]
                        nc.vector.tensor_scalar(out=sl, in0=heads_in[i],
                                                scalar1=scal_aps[i], scalar2=None,
                                                op0=Alu.add)
                        nc.vector.tensor_tensor(out=sl, in0=sl, in1=mask_t,
                                                op=Alu.add)
                    p = work.tile([128, RSH * 2], F16, tag="p")
                    nc.vector.tensor_scalar(out=p, in0=u, scalar1=ALPHA,
                                            scalar2=None, op0=Alu.mult)
                    nc.vector.tensor_tensor(out=u, in0=u, in1=p, op=Alu.max)
                    nc.scalar.activation(out=p, in_=u, func=Act.Exp)
                    return p
                p = work.tile([128, RSH * 2], F16, tag="p")
                nc.scalar.activation(out=p, in_=u, func=Act.Exp)
                return p

            with tc.tile_pool(name="ps_main", bufs=1, space="PSUM") as ps_main:
                # PSUM is 8 banks = 16KB/partition; a [128,2048] f32 pair tile
                # is 4 banks.  Two alternating 1-buf tags give double-buffering
                # within budget, and the later phases reuse the same rings.
                # ---------------- layer 1 ----------------
                for hp in range(H // 2):
                    ha, hb = 2 * hp, 2 * hp + 1
                    ph1 = ps_main.tile([128, RSH * 2], F32,
                                       tag="pa" if hp % 2 == 0 else "pb",
                                       name=f"ph1_{hp}")
                    for cb in range(NCB):
                        u = work.tile([128, RSH * 2], F16, tag="u")
                        p = build_site(hp * NCB + cb, u,
                                       [f2sb[:, cb * H + ha:cb * H + ha + 1],
                                        f2sb[:, cb * H + hb:cb * H + hb + 1]],
                                       madj[cb], [f1rep[ha], f1rep[hb]])
                        if DBG and hp == 0 and cb == 0:
                            nc.sync.dma_start(out=dbg["dbg_p00"][:, :], in_=p)
                            nc.sync.dma_start(out=dbg["dbg_u00"][:, :], in_=u)
                        for i in range(2):
                            h = ha + i
                            for j in range(2):
                                nc.tensor.matmul(
                                    ph1[:, i * RSH + j * 512:i * RSH + (j + 1) * 512],
                                    lhsT=whaug[cb][:, h * 128:(h + 1) * 128],
                                    rhs=p[:, i * RSH + j * 512:i * RSH + (j + 1) * 512],
                                    start=(cb == 0), stop=(cb == NCB - 1))
                    # epilogue, pair-batched on [64, 2048]:
                    # rows 64:128 of each head block hold the replicated row-sum.
                    # recip/shift/hn pipelined in 1024-col quarters to shorten
                    # the serial chain after the last matmul.
                    if DBG and hp == 0:
                        pcp = ep.tile([128, RSH * 2], F32, tag="rinv")
                        nc.vector.tensor_copy(out=pcp, in_=ph1)
                        nc.sync.dma_start(out=dbg["dbg_ph1_0"][:, :], in_=pcp)
                    rinv = ep.tile([128, RSH * 2], F32, tag="rinv")
                    hn = ep.tile([64, RSH * 2], F16, tag="hn")
                    for q4 in range(2):
                        qs = slice(q4 * RSH, (q4 + 1) * RSH)
                        # full-partition AP: reciprocal_approx_fast mis-addresses
                        # partition-offset slices (rows 0:64 are don't-care)
                        nc.vector.reciprocal_approx_fast(out=rinv[:, qs],
                                                         in_=ph1[:, qs])
                        nc.sync.dma_start(out=rinv[0:64, qs], in_=rinv[64:128, qs])
                        nc.vector.tensor_tensor(out=hn[:, qs], in0=ph1[0:64, qs],
                                                in1=rinv[0:64, qs], op=Alu.mult)
                    # ELU: q = exp(min(hn,0)); h' = max(q-1, hn)
                    q = ep.tile([64, RSH * 2], F16, tag="q")
                    nc.vector.tensor_scalar(out=q, in0=hn, scalar1=0.0,
                                            scalar2=None, op0=Alu.min)
                    nc.scalar.activation(out=q, in_=q, func=Act.Exp)
                    nc.vector.tensor_scalar(out=q, in0=q, scalar1=-1.0,
                                            scalar2=None, op0=Alu.add)
                    nc.vector.tensor_tensor(out=hT[hp][0:64, :], in0=q[:, 0:RSH],
                                            in1=hn[:, 0:RSH], op=Alu.max)
                    tmp = ep.tile([64, RSH], F16, tag="hodd")
                    nc.vector.tensor_tensor(out=tmp, in0=q[:, RSH:],
                                            in1=hn[:, RSH:], op=Alu.max)
                    nc.sync.dma_start(out=hT[hp][64:128, :], in_=tmp)
                    if DBG and hp == 0:
                        nc.sync.dma_start(out=dbg["dbg_rinv0"][:, :], in_=rinv)
                        nc.sync.dma_start(out=dbg["dbg_hn0"][:, :], in_=hn)

                # ---------------- Wh2 + exchange ----------------
                ccsb = res.tile([128, 8 * 33], F16, tag="ccsb")
                for half in range(2):
                    pw2 = ps_main.tile([128, 4 * 33], F32,
                                       tag="pa" if half == 0 else "pb")
                    for nbq in range(4):
                        nb = half * 4 + nbq
                        for k in range(4):
                            nc.tensor.matmul(pw2[:, nbq * 33:(nbq + 1) * 33],
                                             lhsT=hT[k][:, nb * 128:(nb + 1) * 128],
                                             rhs=woaug[k], start=(k == 0), stop=(k == 3))
                    nc.vector.tensor_copy(
                        out=ccsb[:, half * 132:(half + 1) * 132], in_=pw2)
                nc.sync.dma_start(
                    out=cc_in[:, :].rearrange("(nb p) j -> p nb j", p=128),
                    in_=ccsb.rearrange("p (nb j) -> p nb j", j=33))
                nc.gpsimd.collective_compute(
                    "AllGather", Alu.bypass, replica_groups=GROUPS,
                    ins=[cc_in[:, :]], outs=[cc_out[:, :]])

                # g1 replicated (no collective dependency): [128, 1024]
                pg1 = ps_main.tile([128, RSH], F32, tag="pa")
                for j in range(2):
                    for k in range(4):
                        nc.tensor.matmul(pg1[:, j * 512:(j + 1) * 512],
                                         lhsT=wo1rep[k],
                                         rhs=hT[k][:, j * 512:(j + 1) * 512],
                                         start=(k == 0), stop=(k == 3))
                g1rep = res.tile([128, RSH], F16, tag="g1rep")
                nc.vector.tensor_copy(out=g1rep, in_=pg1)
                # prebuild L2 u1 = g1 + mask for the first 6 sites while the
                # AllGather is in flight (no cc dependency -> DVE stays busy);
                # the last 2 build inline post-cc, freeing 8KB/partition of
                # SBUF for a deeper work-tile ring.
                N_PRE = 6
                l2u = []
                for cbp in range(N_PRE):
                    u2 = work.tile([128, RSH * 2], F16, tag="l2u", bufs=N_PRE,
                                   name=f"l2u{cbp}")
                    for i, cc in enumerate((2 * cbp, 2 * cbp + 1)):
                        nc.vector.tensor_tensor(out=u2[:, i * RSH:(i + 1) * RSH],
                                                in0=g1rep, in1=madj[cc], op=Alu.add)
                    l2u.append(u2)
                # wh2[cb]: [Wh2(32) | ones(32)]; g2 lands in one [128,16] tile
                wh2 = [res.tile([128, 64], F16, tag=f"wh2_{cb}", name=f"wh2_{cb}") for cb in range(NCB)]
                cc_out_r = cc_out[:, :].rearrange("(cb p) j -> p cb j", p=128)
                g2f16 = res.tile([128, NCB], F16, tag="g2f16")
                nc.sync.dma_start(out=g2f16, in_=cc_out_r[:, :, 32])
                g2sb = res.tile([128, NCB], F32, tag="g2sb")
                nc.vector.tensor_copy(out=g2sb, in_=g2f16)
                for cb in range(NCB):
                    nc.sync.dma_start(out=wh2[cb][:, 0:32], in_=cc_out_r[:, cb, 0:32])
                    nc.gpsimd.memset(wh2[cb][:, 32:64], 1.0)

                if DBG:
                    nc.sync.dma_start(out=dbg["dbg_hT0"][:, :], in_=hT[0])
                    nc.sync.dma_start(out=dbg["dbg_hT3"][:, :], in_=hT[3])
                    nc.sync.dma_start(out=dbg["dbg_g1rep"][:, :], in_=g1rep)
                    nc.sync.dma_start(out=dbg["dbg_wh2_0"][:, :], in_=wh2[0])
                    nc.sync.dma_start(out=dbg["dbg_g2sb"][:, :], in_=g2sb)
                    nc.sync.dma_start(out=dbg["dbg_f1rep0"][:, :], in_=f1rep[0])
                    nc.sync.dma_start(out=dbg["dbg_f2sb"][:, :], in_=f2sb)
                    nc.sync.dma_start(out=dbg["dbg_whaug0"][:, :], in_=whaug[0])

                # ---------------- layer 2 ----------------
                po = ps_main.tile([128, RSH], F32, tag="pb")
                for cbp in range(NCB // 2):
                    ca, cb2 = 2 * cbp, 2 * cbp + 1
                    if cbp < N_PRE:
                        u2 = l2u[cbp]
                    else:
                        u2 = work.tile([128, RSH * 2], F16, tag="u")
                        for i, cc in enumerate((ca, cb2)):
                            nc.vector.tensor_tensor(
                                out=u2[:, i * RSH:(i + 1) * RSH],
                                in0=g1rep, in1=madj[cc], op=Alu.add)
                    path = SCHED[(64 + cbp) % len(SCHED)]
                    if path == "A":
                        for i, cc in enumerate((ca, cb2)):
                            sl = u2[:, i * RSH:(i + 1) * RSH]
                            nc.scalar.activation(out=sl, in_=sl, func=Act.Prelu,
                                                 bias=g2sb[:, cc:cc + 1], alpha=ALPHA)
                        p2 = work.tile([128, RSH * 2], F16, tag="p")
                        nc.scalar.activation(out=p2, in_=u2, func=Act.Exp)
                    else:
                        for i, cc in enumerate((ca, cb2)):
                            sl = u2[:, i * RSH:(i + 1) * RSH]
                            nc.vector.tensor_scalar(out=sl, in0=sl,
                                                    scalar1=g2sb[:, cc:cc + 1],
                                                    scalar2=None, op0=Alu.add)
                        p2 = work.tile([128, RSH * 2], F16, tag="p")
                        nc.vector.tensor_scalar(out=p2, in0=u2, scalar1=ALPHA,
                                                scalar2=None, op0=Alu.mult)
                        nc.vector.tensor_tensor(out=u2, in0=u2, in1=p2, op=Alu.max)
                        nc.scalar.activation(out=p2, in_=u2, func=Act.Exp)
                    for i, cc in enumerate((ca, cb2)):
                        for j in range(2):
                            nc.tensor.matmul(
                                po[0:64, j * 512:(j + 1) * 512],
                                lhsT=wh2[cc],
                                rhs=p2[:, i * RSH + j * 512:i * RSH + (j + 1) * 512],
                                start=(cc == 0), stop=(cc == NCB - 1))
                # L2 epilogue: rows 32:64 hold replicated sums
                rinv2 = ep.tile([64, RSH], F32, tag="rinv2")
                ov = ep.tile([32, RSH], F32, tag="ov")
                osb = ep.tile([32, RSH], F32, tag="osb")
                nc.vector.reciprocal_approx_fast(out=rinv2, in_=po[0:64, :])
                nc.sync.dma_start(out=rinv2[0:32, :], in_=rinv2[32:64, :])
                nc.vector.tensor_tensor(out=ov, in0=po[0:32, :],
                                        in1=rinv2[0:32, :], op=Alu.mult)
                nc.vector.tensor_scalar(out=osb, in0=ov, scalar1=OUT_SLOPE,
                                        scalar2=None, op0=Alu.mult)
                nc.vector.tensor_tensor(out=osb, in0=osb, in1=ov, op=Alu.max)
                nc.sync.dma_start(out=out_e[:, :], in_=osb)
    from concourse.library_overlay import lower_extended_insts
    lower_extended_insts(nc)
    _legalize_multi_waits(nc)
    return nc


_NC = None


def _host_prep(x, adj, W, a1, a2, Wout, ao1, ao2):
    x = np.asarray(x, dtype=np.float32)
    adj = np.asarray(adj, dtype=np.float32)
    W = np.asarray(W, dtype=np.float32)
    a1 = np.asarray(a1, dtype=np.float32)
    a2 = np.asarray(a2, dtype=np.float32)
    Wout = np.asarray(Wout, dtype=np.float32)
    ao1 = np.asarray(ao1, dtype=np.float32)
    ao2 = np.asarray(ao2, dtype=np.float32)

    xT = np.ascontiguousarray(x.transpose(0, 2, 1))                # [B,128,N]
    madj = ((adj.transpose(0, 2, 1) - 1.0) * (-MASK)).astype(np.float16)  # 0 / -150, [B,N,N] as (c,r)
    wall = np.ascontiguousarray(W.transpose(1, 0, 2).reshape(F_IN, H * F_HID))
    wa1 = np.einsum('hfo,ho->fh', W, a1)                           # [128,H]
    wa2 = np.ascontiguousarray(np.einsum('hfo,ho->fh', W, a2))     # [128,H]
    w1rep = np.repeat(wa1, 128, axis=1)                            # [128,H*128]
    wo1 = Wout @ ao1                                               # [512]
    wo2 = Wout @ ao2
    woaug = np.zeros((H * F_HID, 33), np.float16)
    woaug[:, :C] = Wout.astype(np.float16)
    woaug[:, 32] = wo2.astype(np.float16)
    wo1rep = np.repeat(wo1[:, None], 128, axis=1).astype(np.float16)

    in_maps = []
    for c in range(8):
        b, s = c // 2, c % 2
        sl = slice(s * RSH, (s + 1) * RSH)
        in_maps.append({
            "xT": np.ascontiguousarray(xT[b]),
            "xtr": np.ascontiguousarray(xT[b][:, sl]),
            "madj": np.ascontiguousarray(madj[b][:, sl]),
            "wall": wall,
            "wa2": wa2,
            "w1rep": np.ascontiguousarray(w1rep),
            "woaug": woaug,
            "wo1rep": wo1rep,
        })
    return in_maps


def run(x, adj, W, a1, a2, Wout, ao1, ao2, trace=False, **trace_kw):
    global _NC
    if _NC is None:
        _NC = build_nc()
    in_maps = _host_prep(x, adj, W, a1, a2, Wout, ao1, ao2)
    r = run_bass_kernel_spmd(_NC, in_maps, list(range(8)), trace=trace, **trace_kw)
    out = np.empty((B, N, C), np.float32)
    for c in range(8):
        b, s = c // 2, c % 2
        out[b, s * RSH:(s + 1) * RSH, :] = r.results[c]["out"].T
    return out, r


def kernel(x, adj, W, a1, a2, Wout, ao1, ao2, batch_size=None):
    out, _ = run(x, adj, W, a1, a2, Wout, ao1, ao2)
    return out
